# revision 39
# baseline (speedup 1.0000x reference)
"""Bahdanau additive-attention kernel for one TRN2 chip (8 NeuronCores).

Reference computation (per batch b):
    q      = dec[b] @ w2 + b2 + b1                      # [1, E]
    H      = enc[b] @ w1                                # [S, E]
    scores = tanh(H + q) @ v (+ bv, softmax-invariant)  # [S, 1]
    attn   = softmax(scores over S)
    out[b] = attn @ enc[b]                              # [E]

Sharding: pure data-parallel over batch. 32 batches / 8 cores = 4 per core.
No collectives. Weights replicated. The host passes enc twice: transposed
([b, e, s]) in fp8-e4m3 for the H matmul, and natural layout in bf16 for the
context reduction. The small q-side operands (dec, b1+b2, v) are pre-
transposed into their on-chip layouts by the host: the device-side gathers
they replaced ran at one descriptor per element (the 1-byte dec gather alone
occupied a DMA engine for ~17us and starved the startup-critical loads).

The dominant H matmul runs in fp8 (e4m3) with MatmulPerfMode.DoubleRow: each
PE instruction contracts TWO 128-row k-chunks (lhsT [128,2,M], rhs [128,2,N])
at fp8's double rate - 2x the bf16/fp32r matmul throughput. w1 and w2 are
pre-scaled by 64 on the host so their [-1/32, 1/32] entries land in e4m3's
normal range; the 1/64 descale folds into the ScalarE tanh / the q bias add.
dec also travels as fp8 (the q matmul needs matching operand dtypes).
Quantization puts the end-to-end relative error at ~1.2e-2 (gate: 2e-2).

Per-core dataflow (B=4, S=2048, E=1024), working H^T = w1^T @ enc^T so the
tanh bias (q) is a per-partition scalar fused into the ScalarE activation.
Engine balance per s-block of 512 (PE 6.9us of H is the floor; DVE and
ScalarE must stay below it):

    PE:      8 H groups (32 fp8-DR matmuls)           ~6.9us
             + 4 ctx rank-1 matmuls (lag-3 block)     ~0.9us
             + 1 v-fold ones-matmul (lag-2 block)     ~0.2us
    ScalarE: 8 tanh + 1 exp                           ~6.2us
    DVE:     v-chain, 8 mult-accumulate ops           ~5.9us
             (scores = v^T tanh, folded into PSUM via the ones-matmul)

  softmax normalization is deferred to one final scale by 1/sum(exp):
  scores are bounded (|tanh|<1, v fixed) so no max-subtraction is needed.
  attn weights go to DRAM and return transposed ([s%128, s/128]) for the
  ctx stationary columns (SBUF->SBUF partition-scatter DMA corrupts).

The context reduction attn^T @ enc runs on the PE for every batch (rank-1
attn-column x natural-enc matmuls, accumulated in a batch-long open PSUM
group): on the DVE it would cost 5.1us/block, tipping the DVE over the PE's
block time and piling ~30us of serial chain into the kernel's drain.

Startup: the first H matmul gates only on w1's first DoubleRow pair and the
first half of encT (dispatched first). The first two s-blocks drain their H
PSUM banks through idle-DVE copies to SBUF (bf16) instead of tanh, so the
opening H stream never waits on the w2 load; the deferred tanhs run from
SBUF once q lands (q is injected into the PE stream between the first two
blocks, right as w2 arrives).

Drain: the last two s-blocks run their v-projection fully on the PE (the
DVE chain plus cross-engine fold would otherwise serialize into the tail).

HW notes learned the hard way (all deterministic, simulator-invisible):
  - the first DMA into an SBUF region reused from earlier-scope tiles, when
    queued near 4-byte-stride gather descriptors, lands with the low 12
    mantissa bits of each aligned word zeroed -> main pools are allocated
    before the setup pool and the first encT tile is DMA'd first;
  - fp8-typed ExternalInput uploads can corrupt; fp8 bytes travel as uint8
    and the DRAM APs are bitcast to fp8 in-kernel;
  - SBUF->SBUF partition-scatter DMA corrupts -> the exp transpose goes
    through DRAM;
  - each dma_start costs ~0.4us of sync-queue dispatch -> multi-chunk
    loads are consolidated into single multi-dim DMAs.
"""

import os
import sys

sys.path.insert(0, "/opt/trn_rl_repo")

import numpy as np  # noqa: E402

import concourse.tile as tile  # noqa: E402
from concourse import bacc, mybir  # noqa: E402
from concourse.bass import ts  # noqa: E402
from concourse.bass_utils import run_bass_kernel_spmd  # noqa: E402

P = 128
N_CORES = 8
B_TOTAL = 32
B = B_TOTAL // N_CORES  # 4 batches per core
S = 2048
E = 1024
EC = E // P  # 8 chunks of the hidden dim
EC2 = EC // 2  # 4 double-chunks (DoubleRow pairs)
SB = 512  # s-block (matmul moving size)
NSB = S // SB  # 4 s-blocks per batch
SK = S // P  # 16 s-chunks of 128 per batch
KSB = SB // P  # 4 s-chunks per s-block
NBLK = B * NSB  # 16 s-blocks total per core

F32 = mybir.dt.float32
F32R = mybir.dt.float32r
BF16 = mybir.dt.bfloat16
F8 = mybir.dt.float8e4  # e4m3
U8 = mybir.dt.uint8  # fp8 bytes travel as uint8: the fp8-typed host->device
# upload path corrupts part of the array; same bytes as uint8 arrive intact

W1_SCALE = 64.0  # host multiplies w1/w2 by this before fp8 quantization

SD = F32R  # storage dtype of the DVE-side dataflow (bitcast f32)
Act = mybir.ActivationFunctionType
DR = mybir.MatmulPerfMode.DoubleRow

# bisection switches (temporary): set to "f32r"/"bf16" to revert a piece
CENC_DT = F32R if os.environ.get("ATTN_CENC") == "f32r" else BF16
H_FP8 = os.environ.get("ATTN_H") != "f32r"
# w2/dec in fp8: halves the startup-critical w2 transfer (2MB -> 1MB).
# Measured end-to-end rel err 1.28e-2 vs 1.21e-2 with bf16 (gate 2e-2).
W2_FP8 = os.environ.get("ATTN_W2") != "bf16"
Q_DT = F32R if os.environ.get("ATTN_Q") == "f32r" else BF16  # non-fp8 w2
# number of leading s-blocks whose PSUM banks drain via DVE copy (tanh
# deferred until q lands)
NCOPY = int(os.environ.get("ATTN_NCOPY", "2"))
# number of trailing s-blocks whose v-projection runs fully on the PE
NVPE = int(os.environ.get("ATTN_NVPE", "2"))


def _f32(ap):
    return ap if ap.dtype is F32 else ap.bitcast(F32)


DEBUG = os.environ.get("ATTN_DEBUG") == "1"


def _build_body(nc, tc, ctx, enc, encT_d, dec_t, w1, b12_t, w2, v_t, out, dbg):
    # ---------------- persistent constants ----------------
    const = ctx.enter_context(tc.tile_pool(name="const", bufs=1))
    dram = ctx.enter_context(tc.tile_pool(name="dram", bufs=2, space="DRAM"))

    qT = const.tile([P, EC, B], F32)  # [p, c, b] = q_full[b, c*128+p]
    ones_f = const.tile([P, 1], F32)
    ones_sd = const.tile([P, 1], SD, name="ones_sd")
    nc.vector.memset(ones_f[:], 1.0)
    nc.vector.tensor_copy(ones_sd[:], ones_f[:])

    # ---------------- main pools ----------------
    # Created BEFORE the setup pool: the first encT DMA must not land in a
    # region previously touched by setup tiles -- on HW that combination
    # deterministically truncated the low mantissa bits of the first encT
    # tile (reduced-precision DMA path).
    encT_pool = ctx.enter_context(tc.tile_pool(name="encT", bufs=3))
    cenc_pool = ctx.enter_context(tc.tile_pool(name="cenc", bufs=6))
    work = ctx.enter_context(tc.tile_pool(name="work", bufs=28))
    hraw_pool = ctx.enter_context(tc.tile_pool(name="hraw", bufs=2 * EC))
    accp = ctx.enter_context(tc.tile_pool(name="accp", bufs=2))
    onep = ctx.enter_context(tc.tile_pool(name="onep", bufs=2))
    ps_h = ctx.enter_context(tc.tile_pool(name="ps_h", bufs=4, space="PSUM"))
    ps_s = ctx.enter_context(tc.tile_pool(name="ps_s", bufs=1, space="PSUM"))
    ps_c = ctx.enter_context(tc.tile_pool(name="ps_c", bufs=1, space="PSUM"))

    def encT_dma(b, sb):
        encT = encT_pool.tile([P, EC, SB], F8 if H_FP8 else SD, tag="encT")
        encT_ap = encT_d[:].bitcast(F8) if H_FP8 else encT_d[:]
        encT_r = encT_ap[b].rearrange("(c p) s -> p c s", p=P)
        nc.sync.dma_start(encT[:], encT_r[:, :, ts(sb, SB)])
        return encT

    # ---- setup (pools stay open: the deferred q issue uses them later) ----
    # Dispatch order is startup-critical: w1 first pair and the first encT
    # half gate the opening matmuls, then the rest of each, then w2. The
    # small pre-transposed q-side operands ride the gpsimd queue.
    if True:
        setup = ctx.enter_context(tc.tile_pool(name="setup", bufs=1))
        setup_ps = ctx.enter_context(
            tc.tile_pool(name="setup_ps", bufs=1, space="PSUM")
        )
        # Each dma_start lands on ONE DMA engine (~82 GB/s): the startup-
        # critical loads are split across several dispatches AND several
        # dispatch queues (sync + the idle ScalarE queue) so the transfers
        # run on parallel engines.
        w1_sb = const.tile([P, EC, E], F8 if H_FP8 else SD)  # w1[c*128+p, e']
        w1_ap = w1[:].bitcast(F8) if H_FP8 else w1[:]
        w1_r = w1_ap.rearrange("(c p) e -> p c e", p=P)
        encT_first = encT_pool.tile(
            [P, EC, SB], F8 if H_FP8 else SD, tag="encT"
        )
        encT_ap0 = encT_d[:].bitcast(F8) if H_FP8 else encT_d[:]
        encT_r0 = encT_ap0[0].rearrange("(c p) s -> p c s", p=P)
        w2_sb = setup.tile([P, EC, E], F8 if W2_FP8 else Q_DT)
        w2_ap = w2[:].bitcast(F8) if W2_FP8 else w2[:]
        w2_r = w2_ap.rearrange("(c p) e -> p c e", p=P)

        # the first matmuls gate on encT[0:2] + w1 pair 0 (and only its
        # first E-columns, per-group): encT leads the sync queue, w1 pair 0
        # is E-split across both queues, the rest streams behind
        nc.sync.dma_start(encT_first[:, 0:2, :], encT_r0[:, 0:2, ts(0, SB)])
        nc.scalar.dma_start(
            w1_sb[:, 0:2, 512:1024], w1_r[:, 0:2, 512:1024]
        )
        nc.sync.dma_start(w1_sb[:, 0:2, 0:512], w1_r[:, 0:2, 0:512])
        nc.scalar.dma_start(w1_sb[:, 2:5, :], w1_r[:, 2:5, :])
        nc.sync.dma_start(encT_first[:, 2:4, :], encT_r0[:, 2:4, ts(0, SB)])
        nc.scalar.dma_start(w1_sb[:, 5:8, :], w1_r[:, 5:8, :])
        nc.sync.dma_start(encT_first[:, 4:8, :], encT_r0[:, 4:8, ts(0, SB)])
        nc.scalar.dma_start(w2_sb[:, 0:4, :], w2_r[:, 0:4, :])
        nc.scalar.dma_start(w2_sb[:, 4:8, :], w2_r[:, 4:8, :])

        # host-pre-transposed q-side operands: straight contiguous copies
        decT = setup.tile([P, EC, B], F8 if W2_FP8 else Q_DT)
        dec_ap = dec_t[:].bitcast(F8) if W2_FP8 else dec_t[:]
        nc.gpsimd.dma_start(decT[:], dec_ap)
        b12T = setup.tile([P, EC], F32)
        nc.gpsimd.dma_start(b12T[:], b12_t[:])
        vT = const.tile([P, EC], SD)  # [p, c] = v[c*128+p, 0]
        nc.gpsimd.dma_start(vT[:], v_t[:].bitcast(SD))
        vT_b = const.tile([P, EC], BF16, name="vT_b")  # v-matmul stationary
        nc.vector.tensor_copy(vT_b[:], _f32(vT[:]))

        # q computed directly in [e'-partition, b] layout: stationary w2
        # chunk, moving decT columns -> PSUM [128, B]; the 1/64 descale and
        # b1+b2 bias fold into one DVE op. Deferred: issued into the PE
        # stream between the first two s-blocks so the opening H matmuls
        # never wait behind the w2 load.
        def issue_q():
            for cp in range(EC):
                q_ps = setup_ps.tile([P, B], F32, tag="q_ps")
                for c in range(EC):
                    nc.tensor.matmul(
                        q_ps[:],
                        w2_sb[:, c, ts(cp, P)],
                        decT[:, c, :],
                        start=(c == 0),
                        stop=(c == EC - 1),
                    )
                if W2_FP8:
                    nc.vector.tensor_scalar(
                        qT[:, cp, :],
                        q_ps[:],
                        1.0 / W1_SCALE,
                        b12T[:, cp : cp + 1],
                        mybir.AluOpType.mult,
                        mybir.AluOpType.add,
                    )
                else:
                    nc.vector.tensor_scalar_add(
                        qT[:, cp, :], q_ps[:], b12T[:, cp : cp + 1]
                    )
            if DEBUG:
                nc.sync.dma_start(dbg["qT"][:], qT[:])

    # Work deferred so the PE never waits on ScalarE output or DMA
    # roundtrips: v-stage flushed two s-blocks later, ctx three.
    pending_v = []
    pending_ctx = []
    pend_tanh = []  # deferred tanhs of the copy-drained startup blocks

    def flush_one(queue):
        if queue:
            queue.pop(0)()

    def issue_tanh(src, cp, b):
        th = work.tile([P, SB], BF16, tag="tanh")
        nc.scalar.activation(
            th[:],
            src[:],
            Act.Tanh,
            bias=qT[:, cp, b : b + 1],
            scale=(1.0 / W1_SCALE) if H_FP8 else 1.0,
        )
        return th

    g = -1  # global s-block counter
    for b in range(B):
        a_dram = dram.tile([1, S], SD, tag="a_dram")
        sums = onep.tile([1, NSB], F32, tag="sums")
        expT = work.tile([P, SK], SD, tag="expT")  # [p, k] = exp[k*128+p]
        recip = onep.tile([1, 1], F32, tag="recip")
        # per-batch state for the context reduction: the PSUM tiles are
        # allocated lazily at the first (lag-3) flush so the pool slot's
        # version order matches program order; "acc" is the per-batch DVE
        # accumulator for the s-chunk offloaded from the PE
        tail_state = {}

        for sb in range(NSB):
            g += 1
            # encT[p, c, j] = enc[b, sb*512+j, c*128+p], from host transpose.
            # The tile for block N+1 is DMA'd while block N computes so the
            # PE never waits on it.
            if g == 0:
                encT = encT_first
            else:
                encT = encT_prefetched  # noqa: F821 (set one iteration ago)
            nb, nsb = (b, sb + 1) if sb + 1 < NSB else (b + 1, 0)
            if nb < B:
                encT_prefetched = encT_dma(nb, nsb)

            # q injected between the first two s-blocks: by then w2 has
            # streamed in behind w1/encT and the PE has had a full block of
            # tanh-free work (the copy-drained groups below).
            if g == 1:
                issue_q()

            # ---- main matmuls: H^T chunks via fp8 DoubleRow ----
            # Each DoubleRow instruction contracts e-chunks (2*c2, 2*c2+1):
            # lhsT [128, 2, 128] and rhs [128, 2, 512] pair along dim 1.
            ths = []
            for cp in range(EC):
                ph = ps_h.tile([P, SB], F32, tag="ph")
                if H_FP8:
                    for c2 in range(EC2):
                        nc.tensor.matmul(
                            ph[:],
                            w1_sb[:, 2 * c2 : 2 * c2 + 2, ts(cp, P)],
                            encT[:, 2 * c2 : 2 * c2 + 2, :],
                            start=(c2 == 0),
                            stop=(c2 == EC2 - 1),
                            perf_mode=DR,
                        )
                else:
                    for c in range(EC):
                        nc.tensor.matmul(
                            ph[:],
                            w1_sb[:, c, ts(cp, P)],
                            encT[:, c, :],
                            start=(c == 0),
                            stop=(c == EC - 1),
                        )
                if g < NCOPY:
                    # copy-drain: free the PSUM bank through the idle DVE
                    # (bf16 copy of H*64), tanh deferred until q lands
                    hraw = hraw_pool.tile([P, SB], BF16, tag="hraw")
                    nc.vector.tensor_copy(hraw[:], ph[:])
                    th = work.tile([P, SB], BF16, tag="tanh")
                    pend_tanh.append((th, hraw, cp, b))
                    ths.append(th)
                else:
                    ths.append(issue_tanh(ph, cp, b))

            if DEBUG and b == 0 and sb <= 1:
                nc.sync.dma_start(dbg["encT"][:][sb], encT[:])

            # prefetch the natural-layout bf16 enc chunks this block's
            # (lag-3) ctx reduction will need; issued after the mains so
            # they stay off the startup-critical DMA window
            enc_b = enc[:][b].rearrange("(k p) e -> p k e", p=P)
            cenc = cenc_pool.tile([P, KSB, E], CENC_DT, tag="cenc")
            nc.sync.dma_start(cenc[:], enc_b[:, ts(sb, KSB), :])
            cencs = [cenc[:, j, :] for j in range(KSB)]

            # deferred startup tanhs, spread so the ScalarE queue never
            # spikes far above its steady per-block tanh load
            if g >= 1:
                n_flush = EC if g == 1 else EC // 2
                for _ in range(min(n_flush, len(pend_tanh))):
                    th, hraw, cp0, b0 = pend_tanh.pop(0)
                    nc.scalar.activation(
                        th[:],
                        hraw[:],
                        Act.Tanh,
                        bias=qT[:, cp0, b0 : b0 + 1],
                        scale=(1.0 / W1_SCALE) if H_FP8 else 1.0,
                    )

            # flushes at end of iteration: the v fold-in ones-matmul
            # depends on the DVE v-chain, so it must sit AFTER all 8 H
            # groups in the PE queue (a mid-block flush stalls the PE)
            if len(pending_v) >= 2:
                flush_one(pending_v)
            if len(pending_ctx) >= 3:
                flush_one(pending_ctx)

            def make_v(
                b=b,
                sb=sb,
                ths=ths,
                sums=sums,
                expT=expT,
                recip=recip,
                a_dram=a_dram,
                all_pe=(g >= NBLK - NVPE),
                pe_tr=(g >= NBLK - 3),
            ):
                def issue():
                    # scores[1, s] = sum_e v[e] * tanh[e, s]: a DVE
                    # mult-accumulate chain whose [128, 512] partial folds
                    # into PSUM via one ones-matmul. For the last blocks
                    # the chain would sit in the kernel's drain tail, so it
                    # runs fully on the PE there.
                    pss = ps_s.tile([1, SB], F32, tag="pss", name="pss")
                    if all_pe:
                        for cp in range(EC):
                            nc.tensor.matmul(
                                pss[:],
                                vT_b[:, cp : cp + 1],
                                ths[cp][:],
                                start=(cp == 0),
                                stop=(cp == EC - 1),
                            )
                    else:
                        acc = None
                        for cp in range(EC):
                            nxt = accp.tile(
                                [P, SB], SD, tag=f"vacc{cp % 2}", name="vacc"
                            )
                            if acc is None:
                                nc.vector.tensor_scalar_mul(
                                    nxt[:],
                                    ths[cp][:],
                                    _f32(vT[:, cp : cp + 1]),
                                )
                            else:
                                nc.vector.scalar_tensor_tensor(
                                    nxt[:],
                                    ths[cp][:],
                                    _f32(vT[:, cp : cp + 1]),
                                    acc[:],
                                    mybir.AluOpType.mult,
                                    mybir.AluOpType.add,
                                )
                            acc = nxt
                        nc.tensor.matmul(
                            pss[:], ones_sd[:], acc[:], start=True, stop=True
                        )
                    # exp + running sums (no max needed: |scores| <= 32)
                    exp_sb = onep.tile([1, SB], SD, tag="exp", name="exp_sb")
                    nc.scalar.activation(
                        exp_sb[:],
                        pss[:],
                        Act.Exp,
                        accum_out=sums[:, sb : sb + 1],
                    )
                    # transpose into expT[p, k] = exp[k*128+p]. Normally via
                    # a DRAM roundtrip (SBUF->SBUF partition-scatter DMA
                    # corrupts); for the drain blocks the ~1.6us roundtrip
                    # latency sits on the critical path, so there it runs as
                    # PE transposes ([1,128] -> [128,1] PSUM columns, with a
                    # [1,1] ones as the moving operand) + one DVE copy.
                    if pe_tr:
                        psT = setup_ps.tile(
                            [P, KSB], F32, tag="q_ps", name="psT"
                        )
                        for j in range(KSB):
                            nc.tensor.transpose(
                                psT[:, j : j + 1],
                                _f32(exp_sb[:])[:, ts(j, P)],
                                ones_f[0:1, 0:1],
                            )
                        nc.vector.tensor_copy(expT[:, ts(sb, KSB)], psT[:])
                    else:
                        nc.sync.dma_start(a_dram[:, ts(sb, SB)], exp_sb[:])
                        nc.sync.dma_start(
                            expT[:, ts(sb, KSB)],
                            a_dram[:][0, ts(sb, SB)].rearrange(
                                "(k p) -> p k", p=P
                            ),
                        )
                    if DEBUG and sb == NSB - 1 and b <= 1:
                        nc.sync.dma_start(dbg["expT"][:][b], _f32(expT[:]))
                    if sb == NSB - 1:
                        # softmax denominator: must be issued AFTER the
                        # final sums write (Tile deps follow program order)
                        ssum = onep.tile([1, 1], F32, tag="ssum", name="ssum")
                        nc.vector.tensor_reduce(
                            ssum[:],
                            sums[:],
                            mybir.AxisListType.X,
                            mybir.AluOpType.add,
                        )
                        nc.vector.reciprocal(recip[:], ssum[:])

                return issue

            def make_ctx(
                b=b,
                sb=sb,
                cencs=cencs,
                expT=expT,
                recip=recip,
                tail_state=tail_state,
                last=(sb == NSB - 1),
            ):
                def issue():
                    # ctx[1, E] += attn-chunk-columns x enc chunks: three
                    # s-chunks as rank-1 PE matmuls into a batch-long open
                    # PSUM group (~0.7us/block of PE), the fourth on a
                    # per-batch DVE accumulator (~1.3us/block) folded in at
                    # batch end -- balancing the two engines' block time.
                    if "psc" not in tail_state:
                        tail_state["psc"] = [
                            ps_c.tile(
                                [1, SB], F32, tag=f"psc{h}", name="psc"
                            )
                            for h in range(E // SB)
                        ]
                    psc_list = tail_state["psc"]
                    expT_b = onep.tile(
                        [P, KSB], BF16, tag="expT_b", name="expT_b"
                    )
                    nc.vector.tensor_copy(
                        expT_b[:], _f32(expT[:, ts(sb, KSB)])
                    )
                    for h in range(E // SB):
                        psc = psc_list[h]
                        for j in range(KSB - 1):
                            nc.tensor.matmul(
                                psc[:],
                                expT_b[:, j : j + 1],
                                cencs[j][:, ts(h, SB)],
                                start=(sb == 0 and j == 0),
                                stop=False,
                            )
                    # the fourth chunk on a per-batch DVE accumulator,
                    # folded in at batch end
                    kk = sb * KSB + (KSB - 1)
                    nxt = accp.tile(
                        [P, E], SD, tag=f"cacc{sb % 2}", name="cacc"
                    )
                    attn_k = _f32(expT[:, kk : kk + 1])
                    if sb == 0:
                        nc.vector.tensor_scalar_mul(
                            nxt[:], cencs[KSB - 1], attn_k
                        )
                    else:
                        nc.vector.scalar_tensor_tensor(
                            nxt[:],
                            cencs[KSB - 1],
                            attn_k,
                            tail_state["acc"][:],
                            mybir.AluOpType.mult,
                            mybir.AluOpType.add,
                        )
                    tail_state["acc"] = nxt
                    if last:
                        acc = tail_state["acc"]
                        for h in range(E // SB):
                            nc.tensor.matmul(
                                psc_list[h][:],
                                ones_sd[:],
                                acc[:, ts(h, SB)],
                                start=False,
                                stop=True,
                            )
                        for h in range(E // SB):
                            ctx_sb = onep.tile(
                                [1, SB], F32, tag="ctx", name="ctx_sb"
                            )
                            nc.scalar.activation(
                                ctx_sb[:],
                                psc_list[h][:],
                                Act.Copy,
                                scale=recip[:],
                            )
                            nc.sync.dma_start(
                                out[:][b : b + 1, ts(h, SB)], ctx_sb[:]
                            )

                return issue

            pending_v.append(make_v())
            pending_ctx.append(make_ctx())

    while pending_v or pending_ctx:
        flush_one(pending_v)
        flush_one(pending_ctx)


def build_nc():
    nc = bacc.Bacc(
        "TRN2", target_bir_lowering=False, debug=False, num_devices=N_CORES
    )
    enc = nc.dram_tensor("encoder_outputs", [B, S, E], CENC_DT, kind="ExternalInput")
    encT_d = nc.dram_tensor(
        "encoder_outputs_t", [B, E, S], U8 if H_FP8 else SD, kind="ExternalInput"
    )
    dec_t = nc.dram_tensor(
        "dec_t", [P, EC, B], U8 if W2_FP8 else Q_DT, kind="ExternalInput"
    )
    w1 = nc.dram_tensor("w1", [E, E], U8 if H_FP8 else SD, kind="ExternalInput")
    b12_t = nc.dram_tensor("b12_t", [P, EC], F32, kind="ExternalInput")
    w2 = nc.dram_tensor(
        "w2", [E, E], U8 if W2_FP8 else Q_DT, kind="ExternalInput"
    )
    v_t = nc.dram_tensor("v_t", [P, EC], F32, kind="ExternalInput")
    out = nc.dram_tensor("out", [B, E], F32, kind="ExternalOutput")
    dbg = {}
    if DEBUG:
        dbg["qT"] = nc.dram_tensor("dbg_qT", [P, EC, B], F32, kind="ExternalOutput")
        dbg["expT"] = nc.dram_tensor("dbg_expT", [2, P, SK], F32, kind="ExternalOutput")
        dbg["encT"] = nc.dram_tensor(
            "dbg_encT", [2, P, EC, SB], F8 if H_FP8 else SD,
            kind="ExternalOutput"
        )

    from contextlib import ExitStack

    with tile.TileContext(nc) as tc:
        with ExitStack() as ctx:
            _build_body(
                nc, tc, ctx, enc, encT_d, dec_t, w1, b12_t, w2, v_t, out, dbg
            )
    nc.compile()
    return nc


_NC_CACHE = None


def _get_nc():
    global _NC_CACHE
    if _NC_CACHE is None:
        _NC_CACHE = build_nc()
    return _NC_CACHE


def make_in_maps(inputs):
    """Host-side prep: shard over batch, quantize (fp8 transposed enc for the
    H matmul, bf16 natural enc for the context stage, fp8 w1/w2 scaled by
    64), and pre-transpose the small q-side operands into their on-chip
    layouts (device-side element gathers starve the DMA engines)."""
    f32 = np.float32
    cenc_np = mybir.dt.np(CENC_DT)
    h_np = mybir.dt.np(F8) if H_FP8 else f32
    q_np = mybir.dt.np(F8) if W2_FP8 else mybir.dt.np(Q_DT)
    enc_all = np.asarray(inputs["encoder_outputs"], dtype=f32)
    enc_bf16 = np.ascontiguousarray(enc_all.astype(cenc_np))
    encT_f8 = np.ascontiguousarray(enc_all.astype(h_np).transpose(0, 2, 1))
    w1_f32 = np.asarray(inputs["w1"], dtype=f32)
    w1_f8 = (w1_f32 * f32(W1_SCALE)).astype(h_np) if H_FP8 else w1_f32
    if H_FP8:
        encT_f8 = encT_f8.view(np.uint8)
        w1_f8 = w1_f8.view(np.uint8)
    w2_f32 = np.asarray(inputs["w2"], dtype=f32)
    if W2_FP8:
        w2_q = (w2_f32 * f32(W1_SCALE)).astype(q_np).view(np.uint8)
    else:
        w2_q = w2_f32.astype(q_np)
    # dec_t[p, c, b] = dec[b, 0, c*128+p]
    dec_f32 = np.asarray(inputs["decoder_output"], dtype=f32)[:, 0, :]
    dec_t_all = np.ascontiguousarray(
        dec_f32.reshape(B_TOTAL, EC, P).transpose(2, 1, 0).astype(q_np)
    )
    if W2_FP8:
        dec_t_all = dec_t_all.view(np.uint8)
    # b12_t[p, c] = (b1 + b2)[c*128+p]; v_t[p, c] = v[c*128+p, 0]
    b12 = (
        np.asarray(inputs["b1"], dtype=f32) + np.asarray(inputs["b2"], dtype=f32)
    )
    b12_t = np.ascontiguousarray(b12.reshape(EC, P).T)
    v_t = np.ascontiguousarray(
        np.asarray(inputs["v"], dtype=f32)[:, 0].reshape(EC, P).T
    )
    in_maps = []
    for i in range(N_CORES):
        sl = slice(i * B, (i + 1) * B)
        in_maps.append(
            {
                "encoder_outputs": np.ascontiguousarray(enc_bf16[sl]),
                "encoder_outputs_t": encT_f8[sl],
                "dec_t": np.ascontiguousarray(dec_t_all[:, :, sl]),
                "w1": w1_f8,
                "b12_t": b12_t,
                "w2": w2_q,
                "v_t": v_t,
            }
        )
    return in_maps


def run(inputs, trace=False):
    """Run on hardware. Returns (output [32, 1024] f32, exec_time_ns or None)."""
    nc = _get_nc()
    in_maps = make_in_maps(inputs)
    res = run_bass_kernel_spmd(
        nc, in_maps, core_ids=list(range(N_CORES)), trace=trace
    )
    out = np.concatenate([np.asarray(r["out"]) for r in res.results], axis=0)
    return out, res.exec_time_ns


def kernel(**inputs):
    out, _ = run(inputs)
    return out


# revision 40
# speedup vs baseline: 1.0247x; 1.0247x over previous
"""Bahdanau additive-attention kernel for one TRN2 chip (8 NeuronCores).

Reference computation (per batch b):
    q      = dec[b] @ w2 + b2 + b1                      # [1, E]
    H      = enc[b] @ w1                                # [S, E]
    scores = tanh(H + q) @ v (+ bv, softmax-invariant)  # [S, 1]
    attn   = softmax(scores over S)
    out[b] = attn @ enc[b]                              # [E]

Sharding: pure data-parallel over batch. 32 batches / 8 cores = 4 per core.
No collectives. Weights replicated. The host passes enc twice: transposed
([b, e, s]) in fp8-e4m3 for the H matmul, and natural layout in bf16 for the
context reduction. The small q-side operands (dec, b1+b2, v) are pre-
transposed into their on-chip layouts by the host: the device-side gathers
they replaced ran at one descriptor per element (the 1-byte dec gather alone
occupied a DMA engine for ~17us and starved the startup-critical loads).

The dominant H matmul runs in fp8 (e4m3) with MatmulPerfMode.DoubleRow: each
PE instruction contracts TWO 128-row k-chunks (lhsT [128,2,M], rhs [128,2,N])
at fp8's double rate - 2x the bf16/fp32r matmul throughput. w1 and w2 are
pre-scaled by 64 on the host so their [-1/32, 1/32] entries land in e4m3's
normal range; the 1/64 descale folds into the ScalarE tanh / the q bias add.
dec also travels as fp8 (the q matmul needs matching operand dtypes).
Quantization puts the end-to-end relative error at ~1.2e-2 (gate: 2e-2).

Per-core dataflow (B=4, S=2048, E=1024), working H^T = w1^T @ enc^T so the
tanh bias (q) is a per-partition scalar fused into the ScalarE activation.
Engine balance per s-block of 512 (PE 6.9us of H is the floor; DVE and
ScalarE must stay below it):

    PE:      8 H groups (32 fp8-DR matmuls)           ~6.9us
             + 4 ctx rank-1 matmuls (lag-3 block)     ~0.9us
             + 1 v-fold ones-matmul (lag-2 block)     ~0.2us
    ScalarE: 8 tanh + 1 exp                           ~6.2us
    DVE:     v-chain, 8 mult-accumulate ops           ~5.9us
             (scores = v^T tanh, folded into PSUM via the ones-matmul)

  softmax normalization is deferred to one final scale by 1/sum(exp):
  scores are bounded (|tanh|<1, v fixed) so no max-subtraction is needed.
  attn weights go to DRAM and return transposed ([s%128, s/128]) for the
  ctx stationary columns (SBUF->SBUF partition-scatter DMA corrupts).

The context reduction attn^T @ enc runs on the PE for every batch (rank-1
attn-column x natural-enc matmuls, accumulated in a batch-long open PSUM
group): on the DVE it would cost 5.1us/block, tipping the DVE over the PE's
block time and piling ~30us of serial chain into the kernel's drain.

Startup: the first H matmul gates only on w1's first DoubleRow pair and the
first half of encT (dispatched first). The first two s-blocks drain their H
PSUM banks through idle-DVE copies to SBUF (bf16) instead of tanh, so the
opening H stream never waits on the w2 load; the deferred tanhs run from
SBUF once q lands (q is injected into the PE stream between the first two
blocks, right as w2 arrives).

Drain: the last two s-blocks run their v-projection fully on the PE (the
DVE chain plus cross-engine fold would otherwise serialize into the tail).

HW notes learned the hard way (all deterministic, simulator-invisible):
  - the first DMA into an SBUF region reused from earlier-scope tiles, when
    queued near 4-byte-stride gather descriptors, lands with the low 12
    mantissa bits of each aligned word zeroed -> main pools are allocated
    before the setup pool and the first encT tile is DMA'd first;
  - fp8-typed ExternalInput uploads can corrupt; fp8 bytes travel as uint8
    and the DRAM APs are bitcast to fp8 in-kernel;
  - SBUF->SBUF partition-scatter DMA corrupts -> the exp transpose goes
    through DRAM;
  - each dma_start costs ~0.4us of sync-queue dispatch -> multi-chunk
    loads are consolidated into single multi-dim DMAs.
"""

import os
import sys

sys.path.insert(0, "/opt/trn_rl_repo")

import numpy as np  # noqa: E402

import concourse.tile as tile  # noqa: E402
from concourse import bacc, mybir  # noqa: E402
from concourse.bass import ts  # noqa: E402
from concourse.bass_utils import run_bass_kernel_spmd  # noqa: E402

P = 128
N_CORES = 8
B_TOTAL = 32
B = B_TOTAL // N_CORES  # 4 batches per core
S = 2048
E = 1024
EC = E // P  # 8 chunks of the hidden dim
EC2 = EC // 2  # 4 double-chunks (DoubleRow pairs)
SB = 512  # s-block (matmul moving size)
NSB = S // SB  # 4 s-blocks per batch
SK = S // P  # 16 s-chunks of 128 per batch
KSB = SB // P  # 4 s-chunks per s-block
NBLK = B * NSB  # 16 s-blocks total per core

F32 = mybir.dt.float32
F32R = mybir.dt.float32r
BF16 = mybir.dt.bfloat16
F8 = mybir.dt.float8e4  # e4m3
U8 = mybir.dt.uint8  # fp8 bytes travel as uint8: the fp8-typed host->device
# upload path corrupts part of the array; same bytes as uint8 arrive intact

W1_SCALE = 64.0  # host multiplies w1/w2 by this before fp8 quantization

SD = F32R  # storage dtype of the DVE-side dataflow (bitcast f32)
Act = mybir.ActivationFunctionType
DR = mybir.MatmulPerfMode.DoubleRow

# bisection switches (temporary): set to "f32r"/"bf16" to revert a piece
CENC_DT = F32R if os.environ.get("ATTN_CENC") == "f32r" else BF16
H_FP8 = os.environ.get("ATTN_H") != "f32r"
# w2/dec in fp8: halves the startup-critical w2 transfer (2MB -> 1MB).
# Measured end-to-end rel err 1.28e-2 vs 1.21e-2 with bf16 (gate 2e-2).
W2_FP8 = os.environ.get("ATTN_W2") != "bf16"
Q_DT = F32R if os.environ.get("ATTN_Q") == "f32r" else BF16  # non-fp8 w2
# number of leading s-blocks whose PSUM banks drain via DVE copy (tanh
# deferred until q lands)
NCOPY = int(os.environ.get("ATTN_NCOPY", "2"))
# number of trailing s-blocks whose v-projection runs fully on the PE
NVPE = int(os.environ.get("ATTN_NVPE", "2"))


def _f32(ap):
    return ap if ap.dtype is F32 else ap.bitcast(F32)


DEBUG = os.environ.get("ATTN_DEBUG") == "1"


def _build_body(nc, tc, ctx, enc, encT_d, dec_t, w1, b12_t, w2, v_t, out, dbg):
    # ---------------- persistent constants ----------------
    const = ctx.enter_context(tc.tile_pool(name="const", bufs=1))
    dram = ctx.enter_context(tc.tile_pool(name="dram", bufs=2, space="DRAM"))

    qT = const.tile([P, EC, B], F32)  # [p, c, b] = q_full[b, c*128+p]
    ones_f = const.tile([P, 1], F32)
    ones_sd = const.tile([P, 1], SD, name="ones_sd")
    nc.vector.memset(ones_f[:], 1.0)
    nc.vector.tensor_copy(ones_sd[:], ones_f[:])

    # ---------------- main pools ----------------
    # Created BEFORE the setup pool: the first encT DMA must not land in a
    # region previously touched by setup tiles -- on HW that combination
    # deterministically truncated the low mantissa bits of the first encT
    # tile (reduced-precision DMA path).
    encT_pool = ctx.enter_context(tc.tile_pool(name="encT", bufs=3))
    cenc_pool = ctx.enter_context(tc.tile_pool(name="cenc", bufs=6))
    work = ctx.enter_context(tc.tile_pool(name="work", bufs=28))
    hraw_pool = ctx.enter_context(tc.tile_pool(name="hraw", bufs=2 * EC))
    accp = ctx.enter_context(tc.tile_pool(name="accp", bufs=2))
    onep = ctx.enter_context(tc.tile_pool(name="onep", bufs=2))
    ps_h = ctx.enter_context(tc.tile_pool(name="ps_h", bufs=4, space="PSUM"))
    ps_s = ctx.enter_context(tc.tile_pool(name="ps_s", bufs=1, space="PSUM"))
    ps_c = ctx.enter_context(tc.tile_pool(name="ps_c", bufs=1, space="PSUM"))

    def encT_dma(b, sb):
        encT = encT_pool.tile([P, EC, SB], F8 if H_FP8 else SD, tag="encT")
        encT_ap = encT_d[:].bitcast(F8) if H_FP8 else encT_d[:]
        encT_r = encT_ap[b].rearrange("(c p) s -> p c s", p=P)
        nc.sync.dma_start(encT[:], encT_r[:, :, ts(sb, SB)])
        return encT

    # ---- setup (pools stay open: the deferred q issue uses them later) ----
    # Dispatch order is startup-critical: w1 first pair and the first encT
    # half gate the opening matmuls, then the rest of each, then w2. The
    # small pre-transposed q-side operands ride the gpsimd queue.
    if True:
        setup = ctx.enter_context(tc.tile_pool(name="setup", bufs=1))
        setup_ps = ctx.enter_context(
            tc.tile_pool(name="setup_ps", bufs=1, space="PSUM")
        )
        # Each dma_start lands on ONE DMA engine (~82 GB/s): the startup-
        # critical loads are split across several dispatches AND several
        # dispatch queues (sync + the idle ScalarE queue) so the transfers
        # run on parallel engines.
        w1_sb = const.tile([P, EC, E], F8 if H_FP8 else SD)  # w1[c*128+p, e']
        w1_ap = w1[:].bitcast(F8) if H_FP8 else w1[:]
        w1_r = w1_ap.rearrange("(c p) e -> p c e", p=P)
        encT_first = encT_pool.tile(
            [P, EC, SB], F8 if H_FP8 else SD, tag="encT"
        )
        encT_ap0 = encT_d[:].bitcast(F8) if H_FP8 else encT_d[:]
        encT_r0 = encT_ap0[0].rearrange("(c p) s -> p c s", p=P)
        w2_sb = setup.tile([P, EC, E], F8 if W2_FP8 else Q_DT)
        w2_ap = w2[:].bitcast(F8) if W2_FP8 else w2[:]
        w2_r = w2_ap.rearrange("(c p) e -> p c e", p=P)

        # the first matmuls gate on encT[0:2] + w1 pair 0 (and only its
        # first E-columns, per-group): encT leads the sync queue, w1 pair 0
        # is E-split across both queues, the rest streams behind
        nc.sync.dma_start(encT_first[:, 0:2, :], encT_r0[:, 0:2, ts(0, SB)])
        nc.scalar.dma_start(
            w1_sb[:, 0:2, 512:1024], w1_r[:, 0:2, 512:1024]
        )
        nc.sync.dma_start(w1_sb[:, 0:2, 0:512], w1_r[:, 0:2, 0:512])
        nc.scalar.dma_start(w1_sb[:, 2:5, :], w1_r[:, 2:5, :])
        nc.sync.dma_start(encT_first[:, 2:4, :], encT_r0[:, 2:4, ts(0, SB)])
        nc.scalar.dma_start(w1_sb[:, 5:8, :], w1_r[:, 5:8, :])
        nc.sync.dma_start(encT_first[:, 4:8, :], encT_r0[:, 4:8, ts(0, SB)])
        nc.scalar.dma_start(w2_sb[:, 0:4, :], w2_r[:, 0:4, :])
        nc.scalar.dma_start(w2_sb[:, 4:8, :], w2_r[:, 4:8, :])

        # host-pre-transposed q-side operands: straight contiguous copies
        decT = setup.tile([P, EC, B], F8 if W2_FP8 else Q_DT)
        dec_ap = dec_t[:].bitcast(F8) if W2_FP8 else dec_t[:]
        nc.gpsimd.dma_start(decT[:], dec_ap)
        b12T = setup.tile([P, EC], F32)
        nc.gpsimd.dma_start(b12T[:], b12_t[:])
        vT = const.tile([P, EC], SD)  # [p, c] = v[c*128+p, 0]
        nc.gpsimd.dma_start(vT[:], v_t[:].bitcast(SD))
        vT_b = const.tile([P, EC], BF16, name="vT_b")  # v-matmul stationary
        nc.vector.tensor_copy(vT_b[:], _f32(vT[:]))

        # q computed directly in [e'-partition, b] layout: stationary w2
        # chunk, moving decT columns -> PSUM [128, B]; the 1/64 descale and
        # b1+b2 bias fold into one DVE op. Deferred: issued into the PE
        # stream between the first two s-blocks so the opening H matmuls
        # never wait behind the w2 load.
        def issue_q():
            for cp in range(EC):
                q_ps = setup_ps.tile([P, B], F32, tag="q_ps")
                for c in range(EC):
                    nc.tensor.matmul(
                        q_ps[:],
                        w2_sb[:, c, ts(cp, P)],
                        decT[:, c, :],
                        start=(c == 0),
                        stop=(c == EC - 1),
                    )
                if W2_FP8:
                    nc.vector.tensor_scalar(
                        qT[:, cp, :],
                        q_ps[:],
                        1.0 / W1_SCALE,
                        b12T[:, cp : cp + 1],
                        mybir.AluOpType.mult,
                        mybir.AluOpType.add,
                    )
                else:
                    nc.vector.tensor_scalar_add(
                        qT[:, cp, :], q_ps[:], b12T[:, cp : cp + 1]
                    )
            if DEBUG:
                nc.sync.dma_start(dbg["qT"][:], qT[:])

    # Work deferred so the PE never waits on ScalarE output or DMA
    # roundtrips: v-stage flushed two s-blocks later, ctx three.
    pending_v = []
    pending_ctx = []
    pend_tanh = []  # deferred tanhs of the copy-drained startup blocks

    def flush_one(queue):
        if queue:
            queue.pop(0)()

    def issue_tanh(src, cp, b):
        th = work.tile([P, SB], BF16, tag="tanh")
        nc.scalar.activation(
            th[:],
            src[:],
            Act.Tanh,
            bias=qT[:, cp, b : b + 1],
            scale=(1.0 / W1_SCALE) if H_FP8 else 1.0,
        )
        return th

    g = -1  # global s-block counter
    for b in range(B):
        a_dram = dram.tile([1, S], SD, tag="a_dram")
        sums = onep.tile([1, NSB], F32, tag="sums")
        expT = work.tile([P, SK], SD, tag="expT")  # [p, k] = exp[k*128+p]
        recip = onep.tile([1, 1], F32, tag="recip")
        # per-batch state for the context reduction: the PSUM tiles are
        # allocated lazily at the first (lag-3) flush so the pool slot's
        # version order matches program order; "acc" is the per-batch DVE
        # accumulator for the s-chunk offloaded from the PE
        tail_state = {}

        for sb in range(NSB):
            g += 1
            # encT[p, c, j] = enc[b, sb*512+j, c*128+p], from host transpose.
            # The tile for block N+1 is DMA'd while block N computes so the
            # PE never waits on it.
            if g == 0:
                encT = encT_first
            else:
                encT = encT_prefetched  # noqa: F821 (set one iteration ago)
            nb, nsb = (b, sb + 1) if sb + 1 < NSB else (b + 1, 0)
            if nb < B:
                encT_prefetched = encT_dma(nb, nsb)

            # q injected between the first two s-blocks: by then w2 has
            # streamed in behind w1/encT and the PE has had a full block of
            # tanh-free work (the copy-drained groups below).
            if g == 1:
                issue_q()

            # ---- main matmuls: H^T chunks via fp8 DoubleRow ----
            # Each DoubleRow instruction contracts e-chunks (2*c2, 2*c2+1):
            # lhsT [128, 2, 128] and rhs [128, 2, 512] pair along dim 1.
            ths = []
            for cp in range(EC):
                ph = ps_h.tile([P, SB], F32, tag="ph")
                if H_FP8:
                    for c2 in range(EC2):
                        nc.tensor.matmul(
                            ph[:],
                            w1_sb[:, 2 * c2 : 2 * c2 + 2, ts(cp, P)],
                            encT[:, 2 * c2 : 2 * c2 + 2, :],
                            start=(c2 == 0),
                            stop=(c2 == EC2 - 1),
                            perf_mode=DR,
                        )
                else:
                    for c in range(EC):
                        nc.tensor.matmul(
                            ph[:],
                            w1_sb[:, c, ts(cp, P)],
                            encT[:, c, :],
                            start=(c == 0),
                            stop=(c == EC - 1),
                        )
                if g < NCOPY:
                    # copy-drain: free the PSUM bank through the idle DVE
                    # (bf16 copy of H*64), tanh deferred until q lands
                    hraw = hraw_pool.tile([P, SB], BF16, tag="hraw")
                    nc.vector.tensor_copy(hraw[:], ph[:])
                    th = work.tile([P, SB], BF16, tag="tanh")
                    pend_tanh.append((th, hraw, cp, b))
                    ths.append(th)
                else:
                    ths.append(issue_tanh(ph, cp, b))

            if DEBUG and b == 0 and sb <= 1:
                nc.sync.dma_start(dbg["encT"][:][sb], encT[:])

            # prefetch the natural-layout bf16 enc chunks this block's
            # (lag-3) ctx reduction will need; issued after the mains so
            # they stay off the startup-critical DMA window
            enc_b = enc[:][b].rearrange("(k p) e -> p k e", p=P)
            cenc = cenc_pool.tile([P, KSB, E], CENC_DT, tag="cenc")
            nc.sync.dma_start(cenc[:], enc_b[:, ts(sb, KSB), :])
            cencs = [cenc[:, j, :] for j in range(KSB)]

            # deferred startup tanhs, spread so the ScalarE queue never
            # spikes far above its steady per-block tanh load
            if g >= 1:
                n_flush = EC if g == 1 else EC // 2
                for _ in range(min(n_flush, len(pend_tanh))):
                    th, hraw, cp0, b0 = pend_tanh.pop(0)
                    nc.scalar.activation(
                        th[:],
                        hraw[:],
                        Act.Tanh,
                        bias=qT[:, cp0, b0 : b0 + 1],
                        scale=(1.0 / W1_SCALE) if H_FP8 else 1.0,
                    )

            # flushes at end of iteration: the v fold-in ones-matmul
            # depends on the DVE v-chain, so it must sit AFTER all 8 H
            # groups in the PE queue (a mid-block flush stalls the PE)
            if len(pending_v) >= 2:
                flush_one(pending_v)
            if len(pending_ctx) >= 3:
                flush_one(pending_ctx)

            def make_v(
                b=b,
                sb=sb,
                ths=ths,
                sums=sums,
                expT=expT,
                recip=recip,
                a_dram=a_dram,
                all_pe=(g >= NBLK - NVPE),
            ):
                def issue():
                    # scores[1, s] = sum_e v[e] * tanh[e, s]: a DVE
                    # mult-accumulate chain whose [128, 512] partial folds
                    # into PSUM via one ones-matmul. For the last blocks
                    # the chain would sit in the kernel's drain tail, so it
                    # runs fully on the PE there.
                    pss = ps_s.tile([1, SB], F32, tag="pss", name="pss")
                    if all_pe:
                        for cp in range(EC):
                            nc.tensor.matmul(
                                pss[:],
                                vT_b[:, cp : cp + 1],
                                ths[cp][:],
                                start=(cp == 0),
                                stop=(cp == EC - 1),
                            )
                    else:
                        acc = None
                        for cp in range(EC):
                            nxt = accp.tile(
                                [P, SB], SD, tag=f"vacc{cp % 2}", name="vacc"
                            )
                            if acc is None:
                                nc.vector.tensor_scalar_mul(
                                    nxt[:],
                                    ths[cp][:],
                                    _f32(vT[:, cp : cp + 1]),
                                )
                            else:
                                nc.vector.scalar_tensor_tensor(
                                    nxt[:],
                                    ths[cp][:],
                                    _f32(vT[:, cp : cp + 1]),
                                    acc[:],
                                    mybir.AluOpType.mult,
                                    mybir.AluOpType.add,
                                )
                            acc = nxt
                        nc.tensor.matmul(
                            pss[:], ones_sd[:], acc[:], start=True, stop=True
                        )
                    # exp + running sums (no max needed: |scores| <= 32)
                    exp_sb = onep.tile([1, SB], SD, tag="exp", name="exp_sb")
                    nc.scalar.activation(
                        exp_sb[:],
                        pss[:],
                        Act.Exp,
                        accum_out=sums[:, sb : sb + 1],
                    )
                    # transpose into expT[p, k] = exp[k*128+p]. Normally via
                    # a DRAM roundtrip (SBUF->SBUF partition-scatter DMA
                    # corrupts); for the drain blocks the ~1.6us roundtrip
                    # latency sits on the critical path, so there it runs as
                    # PE transposes ([1,128] -> [128,1] PSUM columns, with a
                    # [1,1] ones as the moving operand) + one DVE copy.
                    if all_pe:
                        psT = setup_ps.tile(
                            [P, KSB], F32, tag="q_ps", name="psT"
                        )
                        for j in range(KSB):
                            nc.tensor.transpose(
                                psT[:, j : j + 1],
                                _f32(exp_sb[:])[:, ts(j, P)],
                                ones_f[0:1, 0:1],
                            )
                        nc.vector.tensor_copy(expT[:, ts(sb, KSB)], psT[:])
                    else:
                        nc.sync.dma_start(a_dram[:, ts(sb, SB)], exp_sb[:])
                        nc.sync.dma_start(
                            expT[:, ts(sb, KSB)],
                            a_dram[:][0, ts(sb, SB)].rearrange(
                                "(k p) -> p k", p=P
                            ),
                        )
                    if DEBUG and sb == NSB - 1 and b <= 1:
                        nc.sync.dma_start(dbg["expT"][:][b], _f32(expT[:]))
                    if sb == NSB - 1:
                        # softmax denominator: must be issued AFTER the
                        # final sums write (Tile deps follow program order)
                        ssum = onep.tile([1, 1], F32, tag="ssum", name="ssum")
                        nc.vector.tensor_reduce(
                            ssum[:],
                            sums[:],
                            mybir.AxisListType.X,
                            mybir.AluOpType.add,
                        )
                        nc.vector.reciprocal(recip[:], ssum[:])

                return issue

            def make_ctx(
                b=b,
                sb=sb,
                cencs=cencs,
                expT=expT,
                recip=recip,
                tail_state=tail_state,
                last=(sb == NSB - 1),
            ):
                def issue():
                    # ctx[1, E] += attn-chunk-columns x enc chunks: three
                    # s-chunks as rank-1 PE matmuls into a batch-long open
                    # PSUM group (~0.7us/block of PE), the fourth on a
                    # per-batch DVE accumulator (~1.3us/block) folded in at
                    # batch end -- balancing the two engines' block time.
                    if "psc" not in tail_state:
                        tail_state["psc"] = [
                            ps_c.tile(
                                [1, SB], F32, tag=f"psc{h}", name="psc"
                            )
                            for h in range(E // SB)
                        ]
                    psc_list = tail_state["psc"]
                    expT_b = onep.tile(
                        [P, KSB], BF16, tag="expT_b", name="expT_b"
                    )
                    nc.vector.tensor_copy(
                        expT_b[:], _f32(expT[:, ts(sb, KSB)])
                    )
                    for h in range(E // SB):
                        psc = psc_list[h]
                        for j in range(KSB - 1):
                            nc.tensor.matmul(
                                psc[:],
                                expT_b[:, j : j + 1],
                                cencs[j][:, ts(h, SB)],
                                start=(sb == 0 and j == 0),
                                stop=False,
                            )
                    # the fourth chunk on a per-batch DVE accumulator,
                    # folded in at batch end
                    kk = sb * KSB + (KSB - 1)
                    nxt = accp.tile(
                        [P, E], SD, tag=f"cacc{sb % 2}", name="cacc"
                    )
                    attn_k = _f32(expT[:, kk : kk + 1])
                    if sb == 0:
                        nc.vector.tensor_scalar_mul(
                            nxt[:], cencs[KSB - 1], attn_k
                        )
                    else:
                        nc.vector.scalar_tensor_tensor(
                            nxt[:],
                            cencs[KSB - 1],
                            attn_k,
                            tail_state["acc"][:],
                            mybir.AluOpType.mult,
                            mybir.AluOpType.add,
                        )
                    tail_state["acc"] = nxt
                    if last:
                        acc = tail_state["acc"]
                        for h in range(E // SB):
                            nc.tensor.matmul(
                                psc_list[h][:],
                                ones_sd[:],
                                acc[:, ts(h, SB)],
                                start=False,
                                stop=True,
                            )
                        for h in range(E // SB):
                            ctx_sb = onep.tile(
                                [1, SB], F32, tag="ctx", name="ctx_sb"
                            )
                            nc.scalar.activation(
                                ctx_sb[:],
                                psc_list[h][:],
                                Act.Copy,
                                scale=recip[:],
                            )
                            nc.sync.dma_start(
                                out[:][b : b + 1, ts(h, SB)], ctx_sb[:]
                            )

                return issue

            pending_v.append(make_v())
            pending_ctx.append(make_ctx())

    while pending_v or pending_ctx:
        flush_one(pending_v)
        flush_one(pending_ctx)


def build_nc():
    nc = bacc.Bacc(
        "TRN2", target_bir_lowering=False, debug=False, num_devices=N_CORES
    )
    enc = nc.dram_tensor("encoder_outputs", [B, S, E], CENC_DT, kind="ExternalInput")
    encT_d = nc.dram_tensor(
        "encoder_outputs_t", [B, E, S], U8 if H_FP8 else SD, kind="ExternalInput"
    )
    dec_t = nc.dram_tensor(
        "dec_t", [P, EC, B], U8 if W2_FP8 else Q_DT, kind="ExternalInput"
    )
    w1 = nc.dram_tensor("w1", [E, E], U8 if H_FP8 else SD, kind="ExternalInput")
    b12_t = nc.dram_tensor("b12_t", [P, EC], F32, kind="ExternalInput")
    w2 = nc.dram_tensor(
        "w2", [E, E], U8 if W2_FP8 else Q_DT, kind="ExternalInput"
    )
    v_t = nc.dram_tensor("v_t", [P, EC], F32, kind="ExternalInput")
    out = nc.dram_tensor("out", [B, E], F32, kind="ExternalOutput")
    dbg = {}
    if DEBUG:
        dbg["qT"] = nc.dram_tensor("dbg_qT", [P, EC, B], F32, kind="ExternalOutput")
        dbg["expT"] = nc.dram_tensor("dbg_expT", [2, P, SK], F32, kind="ExternalOutput")
        dbg["encT"] = nc.dram_tensor(
            "dbg_encT", [2, P, EC, SB], F8 if H_FP8 else SD,
            kind="ExternalOutput"
        )

    from contextlib import ExitStack

    with tile.TileContext(nc) as tc:
        with ExitStack() as ctx:
            _build_body(
                nc, tc, ctx, enc, encT_d, dec_t, w1, b12_t, w2, v_t, out, dbg
            )
    nc.compile()
    return nc


_NC_CACHE = None


def _get_nc():
    global _NC_CACHE
    if _NC_CACHE is None:
        _NC_CACHE = build_nc()
    return _NC_CACHE


def make_in_maps(inputs):
    """Host-side prep: shard over batch, quantize (fp8 transposed enc for the
    H matmul, bf16 natural enc for the context stage, fp8 w1/w2 scaled by
    64), and pre-transpose the small q-side operands into their on-chip
    layouts (device-side element gathers starve the DMA engines)."""
    f32 = np.float32
    cenc_np = mybir.dt.np(CENC_DT)
    h_np = mybir.dt.np(F8) if H_FP8 else f32
    q_np = mybir.dt.np(F8) if W2_FP8 else mybir.dt.np(Q_DT)
    enc_all = np.asarray(inputs["encoder_outputs"], dtype=f32)
    enc_bf16 = np.ascontiguousarray(enc_all.astype(cenc_np))
    encT_f8 = np.ascontiguousarray(enc_all.astype(h_np).transpose(0, 2, 1))
    w1_f32 = np.asarray(inputs["w1"], dtype=f32)
    w1_f8 = (w1_f32 * f32(W1_SCALE)).astype(h_np) if H_FP8 else w1_f32
    if H_FP8:
        encT_f8 = encT_f8.view(np.uint8)
        w1_f8 = w1_f8.view(np.uint8)
    w2_f32 = np.asarray(inputs["w2"], dtype=f32)
    if W2_FP8:
        w2_q = (w2_f32 * f32(W1_SCALE)).astype(q_np).view(np.uint8)
    else:
        w2_q = w2_f32.astype(q_np)
    # dec_t[p, c, b] = dec[b, 0, c*128+p]
    dec_f32 = np.asarray(inputs["decoder_output"], dtype=f32)[:, 0, :]
    dec_t_all = np.ascontiguousarray(
        dec_f32.reshape(B_TOTAL, EC, P).transpose(2, 1, 0).astype(q_np)
    )
    if W2_FP8:
        dec_t_all = dec_t_all.view(np.uint8)
    # b12_t[p, c] = (b1 + b2)[c*128+p]; v_t[p, c] = v[c*128+p, 0]
    b12 = (
        np.asarray(inputs["b1"], dtype=f32) + np.asarray(inputs["b2"], dtype=f32)
    )
    b12_t = np.ascontiguousarray(b12.reshape(EC, P).T)
    v_t = np.ascontiguousarray(
        np.asarray(inputs["v"], dtype=f32)[:, 0].reshape(EC, P).T
    )
    in_maps = []
    for i in range(N_CORES):
        sl = slice(i * B, (i + 1) * B)
        in_maps.append(
            {
                "encoder_outputs": np.ascontiguousarray(enc_bf16[sl]),
                "encoder_outputs_t": encT_f8[sl],
                "dec_t": np.ascontiguousarray(dec_t_all[:, :, sl]),
                "w1": w1_f8,
                "b12_t": b12_t,
                "w2": w2_q,
                "v_t": v_t,
            }
        )
    return in_maps


def run(inputs, trace=False):
    """Run on hardware. Returns (output [32, 1024] f32, exec_time_ns or None)."""
    nc = _get_nc()
    in_maps = make_in_maps(inputs)
    res = run_bass_kernel_spmd(
        nc, in_maps, core_ids=list(range(N_CORES)), trace=trace
    )
    out = np.concatenate([np.asarray(r["out"]) for r in res.results], axis=0)
    return out, res.exec_time_ns


def kernel(**inputs):
    out, _ = run(inputs)
    return out
